# revision 6
# baseline (speedup 1.0000x reference)
"""Trainium2 Bass kernel for the ArchitecturallyCorrectRNN sparse-matmul RNN step.

Computation (see the problem's reference):
    x = concat([a_t, s_t], -1)                  # [B, N_COLS]
    z = segment_sum(values * x.T[cols], rows)   # [N_ROWS, B]
    act = sigmoid(z)
    out1 = act[:N].T                            # [B, N]
    out2 = 2*act[N:].T - 1                      # [B, OUT]

Strategy: the sparse matrix decomposes into
  - Wa  [N, N]      : banded, row i -> cols (i+1 .. i+k) mod N  (k<=129 supported)
  - Ws  [N, STATE]  : random sparse -> densified
  - Ma  [OUT, N]    : random sparse -> densified
  - Ms  [OUT, STATE]: random sparse -> densified
Host densifies these (scatter-add via bincount), pre-transposes everything into
[128, X] SBUF-layout panels, and row-shards z across the 8 NeuronCores
(1024 band rows per core). The band becomes 2 accumulating [128x128]x[128x256]
matmuls per 128-row chunk; Ws adds 4 more. out2's contraction over N is
sharded across cores; the [64, 256] partials are summed on the host during
unshard. Sigmoid runs on-device (ScalarEngine).

If the indices do not match the band structure, a generic dense row-sharded
fallback kernel is used instead (correct for any COO pattern of these shapes).
"""

import os
import sys

for _p in ("/opt/trn_rl_repo", "/root/.axon_site/_ro/trn_rl_repo"):
    if os.path.isdir(_p) and _p not in sys.path:
        sys.path.insert(0, _p)

import numpy as np

N = 8192
STATE = 512
OUT = 64
B = 256
N_ROWS = N + OUT
N_COLS = N + STATE
NCORES = 8
RPC = N // NCORES          # 1024 band rows per core
MCH = RPC // 128           # 8 row chunks of 128 per core
D_PAD = 129                # max supported band offset; window = 128+D_PAD-1 < 256
ACH = MCH + 1              # a-slice chunks per core (band window spills 1 chunk)
KS = STATE // 128          # 4 contraction chunks over s

# fallback dense-path constants
FB_KD = N_COLS // 128      # 68 contraction chunks
FB_MCH = 9                 # row chunks per core (9*128*8 = 9216 >= N_ROWS)
FB_RPAD = FB_MCH * 128 * NCORES

_GRAPH_CACHE = {}


def _dt_mode():
    return os.environ.get("BASS_KERNEL_DT", "bf16")


def _np_dt(mybir, dt):
    return mybir.dt.np(dt)


def _build_structured_graph(dt_mode):
    """Pipelined structured kernel.

    DMAs are issued at pair-of-row-chunks granularity, spread across the
    sync/vector/gpsimd sequencers (each dma_start costs ~600 ns of
    descriptor-gen on its issuing sequencer), in PE consumption order so
    matmuls start as soon as the first pair's panels land. The out2
    partial-sum matmuls interleave with the z chunks (separate PSUM bank).
    """
    import concourse.bacc as bacc
    import concourse.bass as bass
    import concourse.tile as tile
    import concourse.mybir as mybir

    f32 = mybir.dt.float32
    DT = mybir.dt.bfloat16 if dt_mode == "bf16" else f32
    mm_cast = (lambda ap: ap.bitcast(mybir.dt.float32r)) if dt_mode == "f32r" \
        else (lambda ap: ap)

    nc = bacc.Bacc("TRN2", target_bir_lowering=False, debug=False,
                   num_devices=NCORES)

    A_ext = nc.dram_tensor("a_panel", [128, ACH * 256], DT, kind="ExternalInput")
    S_ext = nc.dram_tensor("s_panel", [128, KS * 256], DT, kind="ExternalInput")
    BD_ext = nc.dram_tensor("band_panel", [128, MCH * 2 * 128], DT,
                            kind="ExternalInput")
    WS_ext = nc.dram_tensor("ws_panel", [128, MCH * KS * 128], DT,
                            kind="ExternalInput")
    MA_ext = nc.dram_tensor("ma_panel", [128, MCH * OUT], DT,
                            kind="ExternalInput")
    MS_ext = nc.dram_tensor("ms_panel", [128, KS * OUT], DT,
                            kind="ExternalInput")
    OZ_ext = nc.dram_tensor("out_z", [128, MCH * 256], f32,
                            kind="ExternalOutput")
    O2_ext = nc.dram_tensor("out_p2", [OUT, 256], f32, kind="ExternalOutput")
    A_d, S_d, BD_d, WS_d, MA_d, MS_d = (t.ap() for t in
                                        (A_ext, S_ext, BD_ext, WS_ext, MA_ext,
                                         MS_ext))
    OZ_d, O2_d = OZ_ext.ap(), O2_ext.ap()

    NPAIR = MCH // 2   # 4 pair-groups of row chunks

    with tile.TileContext(nc) as tc:
        with tc.tile_pool(name="w", bufs=1) as wp, \
             tc.tile_pool(name="ps", bufs=4, space=bass.MemorySpace.PSUM) as pp, \
             tc.tile_pool(name="ps2", bufs=1, space=bass.MemorySpace.PSUM) as pp2, \
             tc.tile_pool(name="o", bufs=1) as op:

            # ---- tile declarations (all resident; SBUF is plentiful) ----
            S = wp.tile([128, KS * 256], DT, tag="S")
            MA = wp.tile([128, MCH * OUT], DT, tag="MA")
            MS = wp.tile([128, KS * OUT], DT, tag="MS")
            # A chunks: [0,1,2] in the prologue tile, then pairs (3,4) (5,6) (7,8)
            A_t = [wp.tile([128, 3 * 256], DT, tag="A0", name="A_0")] + \
                  [wp.tile([128, 2 * 256], DT, tag=f"A{g}", name=f"A_{g}") for g in (1, 2, 3)]

            def A_sl(q):
                if q < 3:
                    return A_t[0][:, 256 * q:256 * (q + 1)]
                g = (q - 3) // 2 + 1
                j = (q - 3) % 2
                return A_t[g][:, 256 * j:256 * (j + 1)]

            BD_t = [wp.tile([128, 4 * 128], DT, tag=f"BD{g}", name=f"BD_{g}")
                    for g in range(NPAIR)]
            WS_t = [wp.tile([128, 8 * 128], DT, tag=f"WS{g}", name=f"WS_{g}")
                    for g in range(NPAIR)]
            OUT_t = [wp.tile([128, 4 * 256], f32, tag=f"OUT{h}", name=f"OUT_{h}")
                     for h in range(2)]

            # ---- prologue DMAs (consumption order, 3 issue sequencers) ----
            nc.sync.dma_start(BD_t[0][:], BD_d[:, 0:4 * 128])
            nc.scalar.dma_start(A_t[0][:], A_d[:, 0:3 * 256])
            nc.gpsimd.dma_start(WS_t[0][:], WS_d[:, 0:8 * 128])
            nc.sync.dma_start(S[:], S_d[:])
            nc.scalar.dma_start(MA[:], MA_d[:])
            nc.gpsimd.dma_start(MS[:], MS_d[:])

            o2 = pp2.tile([OUT, 256], f32, tag="o2")

            for g in range(NPAIR):
                # prefetch pair g+1 while computing pair g
                if g + 1 < NPAIR:
                    gn = g + 1
                    nc.sync.dma_start(BD_t[gn][:],
                                      BD_d[:, 4 * 128 * gn:4 * 128 * (gn + 1)])
                    nc.sync.dma_start(A_t[gn][:],
                                        A_d[:, 256 * (2 * gn + 1):256 * (2 * gn + 3)])
                    nc.gpsimd.dma_start(WS_t[gn][:],
                                        WS_d[:, 8 * 128 * gn:8 * 128 * (gn + 1)])
                for j in range(2):
                    m = 2 * g + j
                    zp = pp.tile([128, 256], f32, tag="z")
                    bd = BD_t[g]
                    nc.tensor.matmul(zp[:],
                                     mm_cast(bd[:, (2 * j) * 128:(2 * j + 1) * 128]),
                                     mm_cast(A_sl(m)),
                                     start=True, stop=False)
                    nc.tensor.matmul(zp[:],
                                     mm_cast(bd[:, (2 * j + 1) * 128:(2 * j + 2) * 128]),
                                     mm_cast(A_sl(m + 1)),
                                     start=False, stop=False)
                    ws = WS_t[g]
                    for k in range(KS):
                        nc.tensor.matmul(zp[:],
                                         mm_cast(ws[:, (4 * j + k) * 128:(4 * j + k + 1) * 128]),
                                         mm_cast(S[:, 256 * k:256 * (k + 1)]),
                                         start=False, stop=(k == KS - 1))
                    # interleaved out2 partial: MA chunk m consumes A chunk m
                    nc.tensor.matmul(o2[:], mm_cast(MA[:, OUT * m:OUT * (m + 1)]),
                                     mm_cast(A_sl(m)),
                                     start=(m == 0), stop=False)
                    ot = OUT_t[g // 2]
                    nc.scalar.activation(ot[:, 256 * (m % 4):256 * (m % 4 + 1)],
                                         zp[:],
                                         mybir.ActivationFunctionType.Sigmoid)
                if g == 1:
                    nc.scalar.dma_start(OZ_d[:, 0:1024], OUT_t[0][:])
                elif g == 3:
                    nc.scalar.dma_start(OZ_d[:, 1024:2048], OUT_t[1][:])

            for k in range(KS):
                nc.tensor.matmul(o2[:], mm_cast(MS[:, OUT * k:OUT * (k + 1)]),
                                 mm_cast(S[:, 256 * k:256 * (k + 1)]),
                                 start=False, stop=(k == KS - 1))
            o2s = op.tile([OUT, 256], f32, tag="o2s")
            nc.vector.tensor_copy(o2s[:], o2[:])
            nc.gpsimd.dma_start(O2_d[:], o2s[:])

    nc.compile()
    return nc


def _build_fallback_graph(dt_mode):
    import concourse.bacc as bacc
    import concourse.bass as bass
    import concourse.tile as tile
    import concourse.mybir as mybir

    f32 = mybir.dt.float32
    DT = mybir.dt.bfloat16 if dt_mode == "bf16" else f32
    mm_cast = (lambda ap: ap.bitcast(mybir.dt.float32r)) if dt_mode == "f32r" \
        else (lambda ap: ap)

    nc = bacc.Bacc("TRN2", target_bir_lowering=False, debug=False,
                   num_devices=NCORES)

    WT_ext = nc.dram_tensor("wt_panel", [128, FB_MCH * FB_KD * 128], DT,
                            kind="ExternalInput")
    XT_ext = nc.dram_tensor("xt_panel", [128, FB_KD * 256], DT,
                            kind="ExternalInput")
    OZ_ext = nc.dram_tensor("out_z", [128, FB_MCH * 256], f32,
                            kind="ExternalOutput")
    WT_d, XT_d, OZ_d = WT_ext.ap(), XT_ext.ap(), OZ_ext.ap()

    with tile.TileContext(nc) as tc:
        with tc.tile_pool(name="x", bufs=1) as xp, \
             tc.tile_pool(name="w", bufs=2) as wp, \
             tc.tile_pool(name="ps", bufs=4, space=bass.MemorySpace.PSUM) as pp, \
             tc.tile_pool(name="o", bufs=4) as op:
            XT = xp.tile([128, FB_KD * 256], DT, tag="XT")
            nc.sync.dma_start(XT[:], XT_d[:])
            for m in range(FB_MCH):
                slab = wp.tile([128, FB_KD * 128], DT, tag="slab")
                nc.sync.dma_start(
                    slab[:], WT_d[:, m * FB_KD * 128:(m + 1) * FB_KD * 128])
                zp = pp.tile([128, 256], f32, tag="z")
                for k in range(FB_KD):
                    nc.tensor.matmul(zp[:], mm_cast(slab[:, k * 128:(k + 1) * 128]),
                                     mm_cast(XT[:, k * 256:(k + 1) * 256]),
                                     start=(k == 0), stop=(k == FB_KD - 1))
                ot = op.tile([128, 256], f32, tag="ot")
                nc.scalar.activation(ot[:], zp[:],
                                     mybir.ActivationFunctionType.Sigmoid)
                nc.sync.dma_start(OZ_d[:, 256 * m:256 * (m + 1)], ot[:])

    nc.compile()
    return nc


def _get_graph(kind, dt_mode):
    key = (kind, dt_mode)
    if key not in _GRAPH_CACHE:
        if kind == "structured":
            _GRAPH_CACHE[key] = _build_structured_graph(dt_mode)
        else:
            _GRAPH_CACHE[key] = _build_fallback_graph(dt_mode)
    return _GRAPH_CACHE[key]


def _panelize(mat, pdim=128):
    """[R, C] -> [pdim, (R//pdim)*C]: chunk q of rows becomes columns
    [q*C, (q+1)*C) with within-chunk row index on the partition axis."""
    R, C = mat.shape
    assert R % pdim == 0
    return np.ascontiguousarray(
        mat.reshape(R // pdim, pdim, C).transpose(1, 0, 2).reshape(pdim, -1))


def _densify(rows, cols, values):
    """Scatter-add the COO into the four dense blocks. Returns
    (band [N, D_PAD] or None, Ws [N, STATE], Ma [OUT, N], Ms [OUT, STATE])."""
    r = rows.astype(np.int64)
    c = cols.astype(np.int64)
    v = values.astype(np.float64)

    wa = (r < N) & (c < N)
    ws = (r < N) & (c >= N)
    ma = (r >= N) & (c < N)
    ms = (r >= N) & (c >= N)

    d = (c[wa] - r[wa]) % N
    band = None
    if d.size == 0 or (d.min() >= 1 and d.max() <= D_PAD):
        band = np.bincount(r[wa] * D_PAD + (d - 1), weights=v[wa],
                           minlength=N * D_PAD).reshape(N, D_PAD)
    Ws = np.bincount(r[ws] * STATE + (c[ws] - N), weights=v[ws],
                     minlength=N * STATE).reshape(N, STATE)
    Ma = np.bincount((r[ma] - N) * N + c[ma], weights=v[ma],
                     minlength=OUT * N).reshape(OUT, N)
    Ms = np.bincount((r[ms] - N) * STATE + (c[ms] - N), weights=v[ms],
                     minlength=OUT * STATE).reshape(OUT, STATE)
    return (None if band is None else band.astype(np.float32)), \
        Ws.astype(np.float32), Ma.astype(np.float32), Ms.astype(np.float32)


def _sigmoid(x):
    out = np.empty_like(x)
    pos = x >= 0
    out[pos] = 1.0 / (1.0 + np.exp(-x[pos]))
    ex = np.exp(x[~pos])
    out[~pos] = ex / (1.0 + ex)
    return out


def _structured_in_maps(a, s, band, Ws, Ma, Ms, np_dt):
    """Build the per-core input panels for the structured kernel."""
    aT = np.ascontiguousarray(a.T)          # [N, B]
    sT_panel = _panelize(np.ascontiguousarray(s.T))      # [128, KS*256]

    # band lhsT blocks: blkT[q, m2] = band[r0+m2, d-1], q = m2 + d - 1
    B64 = band.reshape(N // 128, 128, D_PAD)
    m2g = np.arange(128)[:, None]
    ddg = np.arange(D_PAD)[None, :]
    qg = np.broadcast_to(m2g + ddg, (128, D_PAD))
    mg = np.broadcast_to(m2g, (128, D_PAD))
    blkT_all = np.zeros((N // 128, 256, 128), np.float32)
    blkT_all[:, qg, mg] = B64

    GaT = np.ascontiguousarray(Ma.T)        # [N, OUT]
    MsT_panel = _panelize(np.ascontiguousarray(Ms.T))    # [128, KS*OUT]
    ms_zero = np.zeros_like(MsT_panel)

    in_maps = []
    for c in range(NCORES):
        idx = (RPC * c + 1 + np.arange(ACH * 128)) % N
        a_panel = _panelize(aT[idx])                     # [128, ACH*256]

        blocks = blkT_all[c * MCH:(c + 1) * MCH]         # [MCH, 256, 128]
        band_panel = _panelize(blocks.reshape(MCH * 256, 128))

        ws_rows = Ws[RPC * c:RPC * (c + 1)]              # [RPC, STATE]
        # chunk (KS*m + k): [p, i2] = Ws[r0 + i2, 128k + p]
        wst = ws_rows.reshape(MCH, 128, KS, 128).transpose(0, 2, 3, 1)
        ws_panel = np.ascontiguousarray(
            wst.reshape(MCH * KS, 128, 128).transpose(1, 0, 2).reshape(128, -1))

        ma_panel = _panelize(GaT[idx[:RPC]])             # [128, MCH*OUT]
        in_maps.append({
            "a_panel": a_panel.astype(np_dt),
            "s_panel": sT_panel.astype(np_dt),
            "band_panel": band_panel.astype(np_dt),
            "ws_panel": ws_panel.astype(np_dt),
            "ma_panel": ma_panel.astype(np_dt),
            "ms_panel": (MsT_panel if c == 0 else ms_zero).astype(np_dt),
        })
    return in_maps


def _fallback_in_maps(a, s, rows, cols, values, np_dt):
    r = rows.astype(np.int64)
    c = cols.astype(np.int64)
    W = np.bincount(r * N_COLS + c, weights=values.astype(np.float64),
                    minlength=N_ROWS * N_COLS).reshape(N_ROWS, N_COLS)
    Wp = np.zeros((FB_RPAD, N_COLS), np.float32)
    Wp[:N_ROWS] = W
    x = np.concatenate([a, s], axis=1)                   # [B, N_COLS]
    xt_panel = _panelize(np.ascontiguousarray(x.T)).astype(np_dt)

    in_maps = []
    rpc = FB_MCH * 128
    for cid in range(NCORES):
        Wc = Wp[cid * rpc:(cid + 1) * rpc]               # [rpc, N_COLS]
        # chunk (m*FB_KD + k): [p, i] = W[r0 + i, 128k + p]
        wt = Wc.reshape(FB_MCH, 128, FB_KD, 128).transpose(0, 2, 3, 1)
        wt_panel = np.ascontiguousarray(
            wt.reshape(FB_MCH * FB_KD, 128, 128).transpose(1, 0, 2)
            .reshape(128, -1))
        in_maps.append({
            "wt_panel": wt_panel.astype(np_dt),
            "xt_panel": xt_panel,
        })
    return in_maps


def _run_spmd(nc, in_maps, trace=False):
    from concourse.bass_utils import run_bass_kernel_spmd
    return run_bass_kernel_spmd(nc, in_maps, core_ids=list(range(NCORES)),
                                trace=trace)


def kernel(a_t, s_t, rows, cols, values, _return_exec_time=False, _trace=False):
    import concourse.mybir as mybir

    a = np.asarray(a_t, np.float32)
    s = np.asarray(s_t, np.float32)
    rows = np.asarray(rows, np.int64)
    cols = np.asarray(cols, np.int64)
    values = np.asarray(values, np.float32)
    assert a.shape == (B, N) and s.shape == (B, STATE), (a.shape, s.shape)

    dt_mode = _dt_mode()
    DTnp = _np_dt(mybir, mybir.dt.bfloat16 if dt_mode == "bf16"
                  else mybir.dt.float32)

    band, Ws, Ma, Ms = _densify(rows, cols, values)

    if band is not None:
        nc = _get_graph("structured", dt_mode)
        in_maps = _structured_in_maps(a, s, band, Ws, Ma, Ms, DTnp)
        res = _run_spmd(nc, in_maps, trace=_trace)
        # out1: per-core out_z [128, MCH*256] chunks -> z.T rows, sigmoid'ed
        zt_parts = [
            res.results[c]["out_z"].reshape(128, MCH, 256)
            .transpose(1, 0, 2).reshape(RPC, 256)
            for c in range(NCORES)
        ]
        out1 = np.ascontiguousarray(np.concatenate(zt_parts, axis=0).T)
        # out2: sum the per-core pre-activation partials, then activate
        z2 = np.sum([res.results[c]["out_p2"] for c in range(NCORES)], axis=0)
        out2 = np.ascontiguousarray((2.0 * _sigmoid(z2) - 1.0).T.astype(np.float32))
    else:
        nc = _get_graph("fallback", dt_mode)
        in_maps = _fallback_in_maps(a, s, rows, cols, values, DTnp)
        res = _run_spmd(nc, in_maps, trace=_trace)
        rpc = FB_MCH * 128
        act_parts = [
            res.results[c]["out_z"].reshape(128, FB_MCH, 256)
            .transpose(1, 0, 2).reshape(rpc, 256)
            for c in range(NCORES)
        ]
        act = np.concatenate(act_parts, axis=0)[:N_ROWS].T   # [B, N_ROWS]
        out1 = np.ascontiguousarray(act[:, :N])
        out2 = np.ascontiguousarray(2.0 * act[:, N:] - 1.0)

    if _return_exec_time:
        return (out1, out2), res
    return out1, out2


# revision 8
# speedup vs baseline: 1.1390x; 1.1390x over previous
"""Trainium2 Bass kernel for the ArchitecturallyCorrectRNN sparse-matmul RNN step.

Computation (see the problem's reference):
    x = concat([a_t, s_t], -1)                  # [B, N_COLS]
    z = segment_sum(values * x.T[cols], rows)   # [N_ROWS, B]
    act = sigmoid(z)
    out1 = act[:N].T                            # [B, N]
    out2 = 2*act[N:].T - 1                      # [B, OUT]

Strategy: the sparse matrix decomposes into
  - Wa  [N, N]      : banded, row i -> cols (i+1 .. i+k) mod N  (k<=129 supported)
  - Ws  [N, STATE]  : random sparse -> densified
  - Ma  [OUT, N]    : random sparse -> densified
  - Ms  [OUT, STATE]: random sparse -> densified
Host densifies these (scatter-add via bincount), pre-transposes everything into
[128, X] SBUF-layout panels, and row-shards z across the 8 NeuronCores
(1024 band rows per core). The band becomes 2 accumulating [128x128]x[128x256]
matmuls per 128-row chunk; Ws adds 4 more. out2's contraction over N is
sharded across cores; the [64, 256] partials are summed on the host during
unshard. Sigmoid runs on-device (ScalarEngine).

If the indices do not match the band structure, a generic dense row-sharded
fallback kernel is used instead (correct for any COO pattern of these shapes).
"""

import os
import sys

for _p in ("/opt/trn_rl_repo", "/root/.axon_site/_ro/trn_rl_repo"):
    if os.path.isdir(_p) and _p not in sys.path:
        sys.path.insert(0, _p)

import numpy as np

N = 8192
STATE = 512
OUT = 64
B = 256
N_ROWS = N + OUT
N_COLS = N + STATE
NCORES = 8
RPC = N // NCORES          # 1024 band rows per core
MCH = RPC // 128           # 8 row chunks of 128 per core
D_PAD = 129                # max supported band offset; window = 128+D_PAD-1 < 256
ACH = MCH + 1              # a-slice chunks per core (band window spills 1 chunk)
KS = STATE // 128          # 4 contraction chunks over s

# fallback dense-path constants
FB_KD = N_COLS // 128      # 68 contraction chunks
FB_MCH = 9                 # row chunks per core (9*128*8 = 9216 >= N_ROWS)
FB_RPAD = FB_MCH * 128 * NCORES

_GRAPH_CACHE = {}


def _dt_mode():
    return os.environ.get("BASS_KERNEL_DT", "bf16")


def _np_dt(mybir, dt):
    return mybir.dt.np(dt)


def _build_structured_graph(dt_mode):
    """Pipelined structured kernel.

    DMAs are issued at pair-of-row-chunks granularity, spread across the
    sync/vector/gpsimd sequencers (each dma_start costs ~600 ns of
    descriptor-gen on its issuing sequencer), in PE consumption order so
    matmuls start as soon as the first pair's panels land. The out2
    partial-sum matmuls interleave with the z chunks (separate PSUM bank).
    """
    import concourse.bacc as bacc
    import concourse.bass as bass
    import concourse.tile as tile
    import concourse.mybir as mybir

    f32 = mybir.dt.float32
    DT = mybir.dt.bfloat16 if dt_mode == "bf16" else f32
    mm_cast = (lambda ap: ap.bitcast(mybir.dt.float32r)) if dt_mode == "f32r" \
        else (lambda ap: ap)

    nc = bacc.Bacc("TRN2", target_bir_lowering=False, debug=False,
                   num_devices=NCORES)

    # packed per-pair panels: P0 = [BD(0:4) | WS(0:8) | A(0:3)], 2304 cols
    #                         Pg = [BD | WS | A(2g+1:2g+3)],      2048 cols
    #                         SMM = [S | MA | MS],                1792 cols
    P_ext = [nc.dram_tensor(f"p{g}_panel", [128, 2304 if g == 0 else 2048], DT,
                            kind="ExternalInput") for g in range(4)]
    SMM_ext = nc.dram_tensor("smm_panel", [128, KS * 256 + MCH * OUT + KS * OUT],
                             DT, kind="ExternalInput")
    OZ_ext = nc.dram_tensor("out_z", [128, MCH * 256], f32,
                            kind="ExternalOutput")
    O2_ext = nc.dram_tensor("out_p2", [OUT, 256], f32, kind="ExternalOutput")
    P_d = [t.ap() for t in P_ext]
    SMM_d, OZ_d, O2_d = SMM_ext.ap(), OZ_ext.ap(), O2_ext.ap()

    NPAIR = MCH // 2   # 4 pair-groups of row chunks

    with tile.TileContext(nc) as tc:
        with tc.tile_pool(name="w", bufs=1) as wp, \
             tc.tile_pool(name="ps", bufs=4, space=bass.MemorySpace.PSUM) as pp, \
             tc.tile_pool(name="ps2", bufs=1, space=bass.MemorySpace.PSUM) as pp2, \
             tc.tile_pool(name="o", bufs=1) as op:

            # ---- tile declarations (all resident; SBUF is plentiful) ----
            P_t = [wp.tile([128, 2304 if g == 0 else 2048], DT, tag=f"P{g}",
                           name=f"P_{g}") for g in range(4)]
            SMM = wp.tile([128, KS * 256 + MCH * OUT + KS * OUT], DT, tag="SMM")
            OUT_t = [wp.tile([128, 4 * 256], f32, tag=f"OUT{h}", name=f"OUT_{h}")
                     for h in range(2)]

            def BD_sl(g, j, kk):       # band lhsT chunk kk of row-chunk m=2g+j
                c0 = (2 * j + kk) * 128
                return P_t[g][:, c0:c0 + 128]

            def WS_sl(g, j, k):        # ws lhsT chunk k of row-chunk m=2g+j
                c0 = 512 + (4 * j + k) * 128
                return P_t[g][:, c0:c0 + 128]

            def A_sl(q):               # a-slice chunk q (rhs)
                if q < 3:
                    return P_t[0][:, 1536 + 256 * q:1536 + 256 * (q + 1)]
                g = (q - 1) // 2
                j = (q - 1) % 2
                return P_t[g][:, 1536 + 256 * j:1536 + 256 * (j + 1)]

            def S_sl(k):
                return SMM[:, 256 * k:256 * (k + 1)]

            def MA_sl(q):
                return SMM[:, 1024 + OUT * q:1024 + OUT * (q + 1)]

            def MS_sl(k):
                return SMM[:, 1536 + OUT * k:1536 + OUT * (k + 1)]

            # ---- all input DMAs up-front on the sync sequencer,
            #      in consumption order; transfers pipeline with compute ----
            nc.sync.dma_start(P_t[0][:], P_d[0][:])
            nc.sync.dma_start(SMM[:], SMM_d[:])
            for g in (1, 2, 3):
                nc.sync.dma_start(P_t[g][:], P_d[g][:])

            o2 = pp2.tile([OUT, 256], f32, tag="o2")

            for g in range(NPAIR):
                for j in range(2):
                    m = 2 * g + j
                    zp = pp.tile([128, 256], f32, tag="z")
                    nc.tensor.matmul(zp[:], mm_cast(BD_sl(g, j, 0)),
                                     mm_cast(A_sl(m)), start=True, stop=False)
                    nc.tensor.matmul(zp[:], mm_cast(BD_sl(g, j, 1)),
                                     mm_cast(A_sl(m + 1)), start=False, stop=False)
                    for k in range(KS):
                        nc.tensor.matmul(zp[:], mm_cast(WS_sl(g, j, k)),
                                         mm_cast(S_sl(k)),
                                         start=False, stop=(k == KS - 1))
                    # interleaved out2 partial: MA chunk m consumes A chunk m
                    nc.tensor.matmul(o2[:], mm_cast(MA_sl(m)), mm_cast(A_sl(m)),
                                     start=(m == 0), stop=False)
                    ot = OUT_t[g // 2]
                    nc.scalar.activation(ot[:, 256 * (m % 4):256 * (m % 4 + 1)],
                                         zp[:],
                                         mybir.ActivationFunctionType.Sigmoid)
                if g == 1:
                    nc.sync.dma_start(OZ_d[:, 0:1024], OUT_t[0][:])
                elif g == 3:
                    nc.sync.dma_start(OZ_d[:, 1024:2048], OUT_t[1][:])

            for k in range(KS):
                nc.tensor.matmul(o2[:], mm_cast(MS_sl(k)), mm_cast(S_sl(k)),
                                 start=False, stop=(k == KS - 1))
            o2s = op.tile([OUT, 256], f32, tag="o2s")
            nc.vector.tensor_copy(o2s[:], o2[:])
            nc.sync.dma_start(O2_d[:], o2s[:])

    nc.compile()
    return nc


def _build_fallback_graph(dt_mode):
    import concourse.bacc as bacc
    import concourse.bass as bass
    import concourse.tile as tile
    import concourse.mybir as mybir

    f32 = mybir.dt.float32
    DT = mybir.dt.bfloat16 if dt_mode == "bf16" else f32
    mm_cast = (lambda ap: ap.bitcast(mybir.dt.float32r)) if dt_mode == "f32r" \
        else (lambda ap: ap)

    nc = bacc.Bacc("TRN2", target_bir_lowering=False, debug=False,
                   num_devices=NCORES)

    WT_ext = nc.dram_tensor("wt_panel", [128, FB_MCH * FB_KD * 128], DT,
                            kind="ExternalInput")
    XT_ext = nc.dram_tensor("xt_panel", [128, FB_KD * 256], DT,
                            kind="ExternalInput")
    OZ_ext = nc.dram_tensor("out_z", [128, FB_MCH * 256], f32,
                            kind="ExternalOutput")
    WT_d, XT_d, OZ_d = WT_ext.ap(), XT_ext.ap(), OZ_ext.ap()

    with tile.TileContext(nc) as tc:
        with tc.tile_pool(name="x", bufs=1) as xp, \
             tc.tile_pool(name="w", bufs=2) as wp, \
             tc.tile_pool(name="ps", bufs=4, space=bass.MemorySpace.PSUM) as pp, \
             tc.tile_pool(name="o", bufs=4) as op:
            XT = xp.tile([128, FB_KD * 256], DT, tag="XT")
            nc.sync.dma_start(XT[:], XT_d[:])
            for m in range(FB_MCH):
                slab = wp.tile([128, FB_KD * 128], DT, tag="slab")
                nc.sync.dma_start(
                    slab[:], WT_d[:, m * FB_KD * 128:(m + 1) * FB_KD * 128])
                zp = pp.tile([128, 256], f32, tag="z")
                for k in range(FB_KD):
                    nc.tensor.matmul(zp[:], mm_cast(slab[:, k * 128:(k + 1) * 128]),
                                     mm_cast(XT[:, k * 256:(k + 1) * 256]),
                                     start=(k == 0), stop=(k == FB_KD - 1))
                ot = op.tile([128, 256], f32, tag="ot")
                nc.scalar.activation(ot[:], zp[:],
                                     mybir.ActivationFunctionType.Sigmoid)
                nc.sync.dma_start(OZ_d[:, 256 * m:256 * (m + 1)], ot[:])

    nc.compile()
    return nc


def _get_graph(kind, dt_mode):
    key = (kind, dt_mode)
    if key not in _GRAPH_CACHE:
        if kind == "structured":
            _GRAPH_CACHE[key] = _build_structured_graph(dt_mode)
        else:
            _GRAPH_CACHE[key] = _build_fallback_graph(dt_mode)
    return _GRAPH_CACHE[key]


def _panelize(mat, pdim=128):
    """[R, C] -> [pdim, (R//pdim)*C]: chunk q of rows becomes columns
    [q*C, (q+1)*C) with within-chunk row index on the partition axis."""
    R, C = mat.shape
    assert R % pdim == 0
    return np.ascontiguousarray(
        mat.reshape(R // pdim, pdim, C).transpose(1, 0, 2).reshape(pdim, -1))


def _densify(rows, cols, values):
    """Scatter-add the COO into the four dense blocks. Returns
    (band [N, D_PAD] or None, Ws [N, STATE], Ma [OUT, N], Ms [OUT, STATE])."""
    r = rows.astype(np.int64)
    c = cols.astype(np.int64)
    v = values.astype(np.float64)

    wa = (r < N) & (c < N)
    ws = (r < N) & (c >= N)
    ma = (r >= N) & (c < N)
    ms = (r >= N) & (c >= N)

    d = (c[wa] - r[wa]) % N
    band = None
    if d.size == 0 or (d.min() >= 1 and d.max() <= D_PAD):
        band = np.bincount(r[wa] * D_PAD + (d - 1), weights=v[wa],
                           minlength=N * D_PAD).reshape(N, D_PAD)
    Ws = np.bincount(r[ws] * STATE + (c[ws] - N), weights=v[ws],
                     minlength=N * STATE).reshape(N, STATE)
    Ma = np.bincount((r[ma] - N) * N + c[ma], weights=v[ma],
                     minlength=OUT * N).reshape(OUT, N)
    Ms = np.bincount((r[ms] - N) * STATE + (c[ms] - N), weights=v[ms],
                     minlength=OUT * STATE).reshape(OUT, STATE)
    return (None if band is None else band.astype(np.float32)), \
        Ws.astype(np.float32), Ma.astype(np.float32), Ms.astype(np.float32)


def _sigmoid(x):
    out = np.empty_like(x)
    pos = x >= 0
    out[pos] = 1.0 / (1.0 + np.exp(-x[pos]))
    ex = np.exp(x[~pos])
    out[~pos] = ex / (1.0 + ex)
    return out


def _structured_in_maps(a, s, band, Ws, Ma, Ms, np_dt):
    """Build the per-core packed input panels for the structured kernel.

    P0  = [BD chunks 0:4 | WS chunks 0:8 | A chunks 0:3]   -> [128, 2304]
    Pg  = [BD 4g:4g+4    | WS 8g:8g+8    | A 2g+1:2g+3]    -> [128, 2048]
    SMM = [S chunks 0:4  | MA chunks 0:8 | MS chunks 0:4]  -> [128, 1792]
    """
    aT = np.ascontiguousarray(a.T)          # [N, B]
    sT_panel = _panelize(np.ascontiguousarray(s.T))      # [128, KS*256]

    # band lhsT blocks: blkT[q, m2] = band[r0+m2, d-1], q = m2 + d - 1
    B64 = band.reshape(N // 128, 128, D_PAD)
    m2g = np.arange(128)[:, None]
    ddg = np.arange(D_PAD)[None, :]
    qg = np.broadcast_to(m2g + ddg, (128, D_PAD))
    mg = np.broadcast_to(m2g, (128, D_PAD))
    blkT_all = np.zeros((N // 128, 256, 128), np.float32)
    blkT_all[:, qg, mg] = B64

    GaT = np.ascontiguousarray(Ma.T)        # [N, OUT]
    MsT_panel = _panelize(np.ascontiguousarray(Ms.T))    # [128, KS*OUT]
    ms_zero = np.zeros_like(MsT_panel)

    in_maps = []
    for c in range(NCORES):
        idx = (RPC * c + 1 + np.arange(ACH * 128)) % N
        a_panel = _panelize(aT[idx])                     # [128, ACH*256]

        blocks = blkT_all[c * MCH:(c + 1) * MCH]         # [MCH, 256, 128]
        band_panel = _panelize(blocks.reshape(MCH * 256, 128))   # [128, 2048]

        ws_rows = Ws[RPC * c:RPC * (c + 1)]              # [RPC, STATE]
        # chunk (KS*m + k): [p, i2] = Ws[r0 + i2, 128k + p]
        wst = ws_rows.reshape(MCH, 128, KS, 128).transpose(0, 2, 3, 1)
        ws_panel = np.ascontiguousarray(
            wst.reshape(MCH * KS, 128, 128).transpose(1, 0, 2).reshape(128, -1))

        ma_panel = _panelize(GaT[idx[:RPC]])             # [128, MCH*OUT]

        m_ = {}
        for g in range(4):
            parts = [band_panel[:, 512 * g:512 * (g + 1)],
                     ws_panel[:, 1024 * g:1024 * (g + 1)]]
            if g == 0:
                parts.append(a_panel[:, 0:768])
            else:
                parts.append(a_panel[:, 256 * (2 * g + 1):256 * (2 * g + 3)])
            m_[f"p{g}_panel"] = np.ascontiguousarray(
                np.concatenate(parts, axis=1)).astype(np_dt)
        m_["smm_panel"] = np.ascontiguousarray(np.concatenate(
            [sT_panel, ma_panel, MsT_panel if c == 0 else ms_zero],
            axis=1)).astype(np_dt)
        in_maps.append(m_)
    return in_maps


def _fallback_in_maps(a, s, rows, cols, values, np_dt):
    r = rows.astype(np.int64)
    c = cols.astype(np.int64)
    W = np.bincount(r * N_COLS + c, weights=values.astype(np.float64),
                    minlength=N_ROWS * N_COLS).reshape(N_ROWS, N_COLS)
    Wp = np.zeros((FB_RPAD, N_COLS), np.float32)
    Wp[:N_ROWS] = W
    x = np.concatenate([a, s], axis=1)                   # [B, N_COLS]
    xt_panel = _panelize(np.ascontiguousarray(x.T)).astype(np_dt)

    in_maps = []
    rpc = FB_MCH * 128
    for cid in range(NCORES):
        Wc = Wp[cid * rpc:(cid + 1) * rpc]               # [rpc, N_COLS]
        # chunk (m*FB_KD + k): [p, i] = W[r0 + i, 128k + p]
        wt = Wc.reshape(FB_MCH, 128, FB_KD, 128).transpose(0, 2, 3, 1)
        wt_panel = np.ascontiguousarray(
            wt.reshape(FB_MCH * FB_KD, 128, 128).transpose(1, 0, 2)
            .reshape(128, -1))
        in_maps.append({
            "wt_panel": wt_panel.astype(np_dt),
            "xt_panel": xt_panel,
        })
    return in_maps


def _run_spmd(nc, in_maps, trace=False):
    from concourse.bass_utils import run_bass_kernel_spmd
    return run_bass_kernel_spmd(nc, in_maps, core_ids=list(range(NCORES)),
                                trace=trace)


def kernel(a_t, s_t, rows, cols, values, _return_exec_time=False, _trace=False):
    import concourse.mybir as mybir

    a = np.asarray(a_t, np.float32)
    s = np.asarray(s_t, np.float32)
    rows = np.asarray(rows, np.int64)
    cols = np.asarray(cols, np.int64)
    values = np.asarray(values, np.float32)
    assert a.shape == (B, N) and s.shape == (B, STATE), (a.shape, s.shape)

    dt_mode = _dt_mode()
    DTnp = _np_dt(mybir, mybir.dt.bfloat16 if dt_mode == "bf16"
                  else mybir.dt.float32)

    band, Ws, Ma, Ms = _densify(rows, cols, values)

    if band is not None:
        nc = _get_graph("structured", dt_mode)
        in_maps = _structured_in_maps(a, s, band, Ws, Ma, Ms, DTnp)
        res = _run_spmd(nc, in_maps, trace=_trace)
        # out1: per-core out_z [128, MCH*256] chunks -> z.T rows, sigmoid'ed
        zt_parts = [
            res.results[c]["out_z"].reshape(128, MCH, 256)
            .transpose(1, 0, 2).reshape(RPC, 256)
            for c in range(NCORES)
        ]
        out1 = np.ascontiguousarray(np.concatenate(zt_parts, axis=0).T)
        # out2: sum the per-core pre-activation partials, then activate
        z2 = np.sum([res.results[c]["out_p2"] for c in range(NCORES)], axis=0)
        out2 = np.ascontiguousarray((2.0 * _sigmoid(z2) - 1.0).T.astype(np.float32))
    else:
        nc = _get_graph("fallback", dt_mode)
        in_maps = _fallback_in_maps(a, s, rows, cols, values, DTnp)
        res = _run_spmd(nc, in_maps, trace=_trace)
        rpc = FB_MCH * 128
        act_parts = [
            res.results[c]["out_z"].reshape(128, FB_MCH, 256)
            .transpose(1, 0, 2).reshape(rpc, 256)
            for c in range(NCORES)
        ]
        act = np.concatenate(act_parts, axis=0)[:N_ROWS].T   # [B, N_ROWS]
        out1 = np.ascontiguousarray(act[:, :N])
        out2 = np.ascontiguousarray(2.0 * act[:, N:] - 1.0)

    if _return_exec_time:
        return (out1, out2), res
    return out1, out2
